# revision 23
# baseline (speedup 1.0000x reference)
"""Paged-attention decode kernel for Trainium2 (Bass/Tile), 8 NeuronCores.

Sharding: one KV head per core (N_KV=8). Each core gets x^T plus its head's
slices of Wq/Wk/Wv/Wo (pre-transposed to DMA-friendly layouts, fp16) and a
host-packed KV stream, computes its 4 query heads' attention and a partial
output projection [B, D]; the host sums the partials.

The KV stream holds only the valid context rows, padded to 128-row chunks.
Chunk layout (256 cols): [K^T (128 cols, partition=d) | V (128 cols,
partition=t%128)].  The whole stream is fetched with a handful of multi-MB
contiguous DMAs into a rotating pool of SBUF tiles.

Per request b with nch chunks:
  QK:  per chunk, K^T chunk is the stationary operand (full 128 cols ->
       fast weight load), q [128,4] moving -> scores [t, g] in PSUM.
  exp: one activation over [128, 4*nch]; garbage rows of the partial last
       chunk are zeroed with a tiny memset so later sums are exact.
  PV:  per chunk, V chunk is stationary (fast weight load), exp-scores
       [128,4] moving -> accumulates att^T [d, g] directly in PSUM (no
       transpose needed later).
  den: one matmul (ones column stationary, scores moving) -> [1, 4*nch],
       then a strided DVE reduce over chunks -> denominators [1,4] written
       into a per-request slice of a shared row.
Normalization is batched at the end: one reciprocal [1,128], one
broadcast matmul (ones-row x rcp -> [128,128]), one elementwise multiply.

The new token's k/v never touch DRAM: its slot inside the last chunk is
patched on device (DVE column copy for K^T, one tiny DMA for the V row).

Everything on the wire is fp16 (measured end-to-end error vs the fp32
reference: ~6e-4); accumulation stays fp32 in PSUM.
"""
import os
import sys
from contextlib import ExitStack

import numpy as np

for _p in ("/opt/trn_rl_repo", "/opt/pypackages"):
    if os.path.isdir(_p) and _p not in sys.path:
        sys.path.append(_p)

import concourse.bass as bass  # noqa: E402,F401
import concourse.tile as tile  # noqa: E402
from concourse import bacc, mybir  # noqa: E402
from concourse.bass_utils import run_bass_kernel_spmd  # noqa: E402

N_HEADS = 32
N_KV = 8
HEAD_DIM = 128
BLOCK_SIZE = 16
MAX_SEQ = 2048
ROPE_BASE = 10000.0
SCALE = HEAD_DIM ** -0.5
B = 32
D = 4096
G = N_HEADS // N_KV   # 4 query heads per kv head
GD = G * HEAD_DIM     # 512
N_CORES = 8
CHW = 2 * HEAD_DIM              # chunk width in the packed KV stream (256)
TILE_CHUNKS = 48                # chunks per SBUF tile
TILE_COLS = TILE_CHUNKS * CHW   # 12288 cols (24 KiB/partition fp16)

F32 = mybir.dt.float32
F16 = mybir.dt.float16

LAST_RESULTS = None  # test harness reads exec_time_ns from here


def _plan(Ls):
    """Greedy-pack requests (in order) into KV tiles of <= TILE_CHUNKS
    chunks. Returns per-request (tile, base_col, nch) and per-tile
    (src_col, cols)."""
    req = []      # b -> (tile, base, nch)
    tiles = []    # tile -> (src_col, cols)
    cur_cols = 0
    src = 0
    for b in range(B):
        nch = (Ls[b] + 127) // 128  # chunks incl. the new-token slot
        w = nch * CHW
        if cur_cols + w > TILE_COLS and cur_cols > 0:
            tiles.append((src, cur_cols))
            src += cur_cols
            cur_cols = 0
        req.append((len(tiles), cur_cols, nch))
        cur_cols += w
    tiles.append((src, cur_cols))
    return req, tiles


def _build_nc(Ls, req_plan, tiles_plan, totc):
    nc = bacc.Bacc("TRN2", target_bir_lowering=False, debug=False,
                   num_devices=N_CORES)

    xt_d = nc.declare_dram_parameter("xT", [128, 32 * B], F16, isOutput=False)
    wq_d = nc.declare_dram_parameter("wq", [128, 32 * GD], F16, isOutput=False)
    wkv_d = nc.declare_dram_parameter("wkv", [128, 32 * 256], F16,
                                      isOutput=False)
    wo_d = nc.declare_dram_parameter("wo", [128, G * D], F16, isOutput=False)
    kv_d = nc.declare_dram_parameter("kv", [128, totc], F16, isOutput=False)
    cq_d = nc.declare_dram_parameter("cq", [B, 64], F32, isOutput=False)
    sq_d = nc.declare_dram_parameter("sq", [B, 64], F32, isOutput=False)
    npad_d = nc.declare_dram_parameter("npad", [1, 128], F32, isOutput=False)
    id_d = nc.declare_dram_parameter("ident", [B, B], F16, isOutput=False)
    out_d = nc.declare_dram_parameter("out", [B, D], F32, isOutput=True)

    with tile.TileContext(nc) as tc, ExitStack() as top:
        cpool = top.enter_context(tc.tile_pool(name="const", bufs=1))
        qT = cpool.tile([128, G * B], F16, tag="qT")     # [d, g*32+b] roped
        knT = cpool.tile([128, B], F16, tag="knT")       # [d, b] roped new k
        vn = cpool.tile([B, 128], F16, tag="vn")         # [b, d] new v
        onescol = cpool.tile([128, 1], F16, tag="ocol")
        onesrow = cpool.tile([1, 128], F16, tag="orow")
        denall = cpool.tile([1, 128], F32, tag="denall")  # [1, b*4+g]
        npadr = cpool.tile([1, 128], F32, tag="npad")
        nc.gpsimd.dma_start(npadr[:], npad_d[:])
        pvraw = cpool.tile([128, 128], F16, tag="pvraw")  # [d, b*4+g] unnorm
        pvTn = cpool.tile([128, 128], F16, tag="pvTn")    # [d, b*4+g] normed
        identH = cpool.tile([B, B], F16, tag="ident")
        nc.vector.memset(onescol[:], 1.0)
        nc.vector.memset(onesrow[:], 1.0)
        nc.gpsimd.dma_start(identH[:], id_d[:])

        kvpool = top.enter_context(tc.tile_pool(name="KV", bufs=3))
        scpool = top.enter_context(tc.tile_pool(name="SC", bufs=3))
        wop = top.enter_context(tc.tile_pool(name="wo", bufs=2))
        kv_tiles = {}
        wo_tiles = []

        def emit_kv(t):
            src, cols = tiles_plan[t]
            kvt = kvpool.tile([128, TILE_COLS], F16, tag="kv", name=f"kv{t}")
            nc.sync.dma_start(kvt[:, 0:cols], kv_d[:, src:src + cols])
            kv_tiles[t] = kvt

        def emit_wo(i):
            wo_t = wop.tile([128, 2 * D], F16, tag="wo", name=f"wo{i}")
            nc.scalar.dma_start(wo_t[:], wo_d[:, i * 2 * D:(i + 1) * 2 * D])
            wo_tiles.append(wo_t)

        emit_kv(0)

        # ---- phase 1: q/k/v projections + rope (row layout [b, d]) -------
        # DMA queues: sync carries xT + the first half of Wq + the KV
        # stream; scalar carries cq/sq/Wkv + the rest of Wq (+ Wo later);
        # gpsimd carries tiny constants and, later, the patches.  K/V
        # projections run first so knT/vn (which gate the patches) are
        # ready early.
        with ExitStack() as s1:
            p1 = s1.enter_context(tc.tile_pool(name="p1", bufs=1))
            wqp = s1.enter_context(tc.tile_pool(name="wqp", bufs=8))
            ps1 = s1.enter_context(
                tc.tile_pool(name="ps1", bufs=1, space="PSUM"))
            tmp = s1.enter_context(tc.tile_pool(name="rtmp", bufs=4))

            xT = p1.tile([128, 32 * B], F16, tag="xT")   # [d, kc*32+b]
            nc.sync.dma_start(xT[:], xt_d[:])
            wq_tiles = []
            for i in range(8):
                wq_t = wqp.tile([128, 4 * GD], F16, tag="wq", name=f"wq{i}")
                eng = nc.sync if i < 4 else nc.scalar
                if i == 4:
                    cq = p1.tile([B, 64], F32, tag="cq")
                    sq = p1.tile([B, 64], F32, tag="sq")
                    nc.scalar.dma_start(cq[:], cq_d[:])
                    nc.scalar.dma_start(sq[:], sq_d[:])
                    wkv_sb = p1.tile([128, 32 * 256], F16, tag="wkv")
                    nc.scalar.dma_start(wkv_sb[:], wkv_d[:])
                eng.dma_start(wq_t[:], wq_d[:, i * 4 * GD:(i + 1) * 4 * GD])
                wq_tiles.append(wq_t)
            if len(tiles_plan) > 1:
                emit_kv(1)

            q_ps = ps1.tile([B, GD], F32, tag="ps_q")     # [b, g*128+d]
            kv_ps = ps1.tile([B, 256], F32, tag="ps_kv")  # [b, k|v]

            # a few q matmuls first (their weights land earliest), then the
            # k/v projections, then the rest of q
            for kc in range(8):
                rx = xT[:, kc * B:(kc + 1) * B]
                nc.tensor.matmul(q_ps[:],
                                 rx, wq_tiles[kc // 4][:, (kc % 4) * GD:
                                                       (kc % 4 + 1) * GD],
                                 start=(kc == 0), stop=False)
            for kc in range(32):
                rx = xT[:, kc * B:(kc + 1) * B]
                nc.tensor.matmul(kv_ps[:],
                                 rx, wkv_sb[:, kc * 256:(kc + 1) * 256],
                                 start=(kc == 0), stop=(kc == 31))
            for kc in range(8, 32):
                rx = xT[:, kc * B:(kc + 1) * B]
                nc.tensor.matmul(q_ps[:],
                                 rx, wq_tiles[kc // 4][:, (kc % 4) * GD:
                                                       (kc % 4 + 1) * GD],
                                 start=False, stop=(kc == 31))

            # rope in row layout: cols [0:64] x1, [64:128] x2 per head
            def rope_row(src, o0, o1):
                t1 = tmp.tile([B, 64], F32, tag="rt1", name="t1")
                t2 = tmp.tile([B, 64], F32, tag="rt2", name="t2")
                nc.vector.tensor_mul(t1[:], src[:, 0:64], cq[:])
                nc.vector.tensor_mul(t2[:], src[:, 64:128], sq[:])
                nc.vector.tensor_sub(o0, t1[:], t2[:])
                t3 = tmp.tile([B, 64], F32, tag="rt1", name="t3")
                t4 = tmp.tile([B, 64], F32, tag="rt2", name="t4")
                nc.vector.tensor_mul(t3[:], src[:, 0:64], sq[:])
                nc.vector.tensor_mul(t4[:], src[:, 64:128], cq[:])
                nc.vector.tensor_add(o1, t3[:], t4[:])

            qr = p1.tile([B, GD], F16, tag="qr")
            knr = p1.tile([B, 128], F16, tag="knr")
            rope_row(kv_ps[:, 0:128], knr[:, 0:64], knr[:, 64:128])
            nc.vector.tensor_copy(vn[:], kv_ps[:, 128:256])
            for g in range(G):
                rope_row(q_ps[:, g * 128:(g + 1) * 128],
                         qr[:, g * 128:g * 128 + 64],
                         qr[:, g * 128 + 64:(g + 1) * 128])

            # transpose q/k_new to [d, b] layouts for the attention matmuls
            ps_t = s1.enter_context(
                tc.tile_pool(name="ps_t", bufs=1, space="PSUM"))
            knT_ps = ps_t.tile([128, B], F16, tag="ps_knT")
            nc.tensor.transpose(knT_ps[:], knr[:], identH[:])
            nc.vector.tensor_copy(knT[:], knT_ps[:])
            qT_ps = ps_t.tile([128, 128], F16, tag="ps_qT")
            for g in range(G):
                nc.tensor.transpose(qT_ps[:, g * B:(g + 1) * B],
                                    qr[:, g * 128:(g + 1) * 128],
                                    identH[:])
            nc.vector.tensor_copy(qT[:], qT_ps[:])

        # ---- phase 2: per-request attention ------------------------------
        with ExitStack() as s3:
            ps_qk = s3.enter_context(
                tc.tile_pool(name="ps_qk", bufs=3, space="PSUM"))
            ps_pv = s3.enter_context(
                tc.tile_pool(name="ps_pv", bufs=2, space="PSUM"))
            ps_d = s3.enter_context(
                tc.tile_pool(name="ps_d", bufs=2, space="PSUM"))

            qks = {}
            rqv = qT[:].rearrange("p (g b) -> p g b", b=B)

            def emit_patch_k(b):
                t, base, nch = req_plan[b]
                kvt = kv_tiles[t]
                lg = Ls[b] - 1
                cb = base + (nch - 1) * CHW
                rnew = lg % 128
                nc.vector.tensor_copy(kvt[:, cb + rnew:cb + rnew + 1],
                                      knT[:, b:b + 1])

            def emit_patch_v(b):
                t, base, nch = req_plan[b]
                kvt = kv_tiles[t]
                lg = Ls[b] - 1
                cb = base + (nch - 1) * CHW
                rnew = lg % 128
                nc.gpsimd.dma_start(
                    kvt[rnew:rnew + 1, cb + 128:cb + 256],
                    vn[b:b + 1, :])

            def emit_qk(b):
                # Pad K columns are zero (plus the patched new-token col),
                # so pad rows score exp(0)=1 exactly; the denominator is
                # corrected once at the end by subtracting the pad counts.
                t, base, nch = req_plan[b]
                kvt = kv_tiles[t]
                rq = rqv[:, :, b]
                qk = ps_qk.tile([128, G * 16], F32, tag="ps_qk",
                                name=f"qk{b}")
                sc = scpool.tile([128, G * 16], F16, tag="SC", name=f"sc{b}")
                for c in range(nch):
                    nc.tensor.matmul(qk[0:128, c * G:(c + 1) * G],
                                     kvt[:, base + c * CHW:base + c * CHW
                                         + 128],
                                     rq, start=True, stop=True)
                nc.scalar.activation(sc[:, 0:G * nch], qk[:, 0:G * nch],
                                     mybir.ActivationFunctionType.Exp,
                                     scale=SCALE)
                qks[b] = sc

            def emit_pv(b):
                t, base, nch = req_plan[b]
                kvt = kv_tiles[t]
                sc = qks.pop(b)
                pv = ps_pv.tile([128, G], F32, tag="ps_pv", name=f"pv{b}")
                for c in range(nch):
                    nc.tensor.matmul(pv[:],
                                     kvt[:, base + c * CHW + 128:
                                         base + c * CHW + 256],
                                     sc[:, c * G:(c + 1) * G],
                                     start=(c == 0), stop=(c == nch - 1))
                d1 = ps_d.tile([1, G * 16], F32, tag="ps_d", name=f"d1{b}")
                nc.tensor.matmul(d1[:, 0:G * nch], onescol[:],
                                 sc[:, 0:G * nch], start=True, stop=True)
                nc.vector.tensor_reduce(
                    denall[:, G * b:G * (b + 1)],
                    d1[:, 0:G * nch].rearrange("p (c g) -> p g c", g=G),
                    mybir.AxisListType.X, mybir.AluOpType.add)
                nc.vector.tensor_copy(pvraw[:, G * b:G * (b + 1)], pv[:])

            # Tile t is processed while t+1..t+2 stream in (3-buf pool).
            # Patches for tile t+1 are emitted at the middle of tile t —
            # by then t+1's DMA has landed, so the in-order DVE/Q7 queues
            # don't stall on it, and tile-granular RAW deps are long
            # resolved when t+1's QKs begin.
            tile_reqs = {}
            for bb in range(B):
                tile_reqs.setdefault(req_plan[bb][0], []).append(bb)
            patched_tiles = set()

            def emit_patches(t):
                if t in patched_tiles or t not in tile_reqs:
                    return
                patched_tiles.add(t)
                for bp in tile_reqs[t]:
                    emit_patch_k(bp)
                for bp in tile_reqs[t]:
                    emit_patch_v(bp)

            cur_t = -1
            pending = []
            for b in range(B):
                t = req_plan[b][0]
                if t > cur_t:
                    while pending:
                        emit_pv(pending.pop(0))
                    if t + 2 < len(tiles_plan) and (t + 2) not in kv_tiles:
                        emit_kv(t + 2)
                    emit_patches(t)  # no-op except for tile 0
                    cur_t = t
                if b in (10, 20):
                    emit_wo((b - 10) // 10)
                mid = tile_reqs[t][(len(tile_reqs[t]) - 1) // 2]
                if b == mid:
                    emit_patches(t + 1)
                emit_qk(b)
                pending.append(b)
                if len(pending) > 2:
                    emit_pv(pending.pop(0))
            while pending:
                emit_pv(pending.pop(0))

            # batched softmax normalization: pvTn = pvraw * (1/den) per col
            rcp = cpool.tile([1, 128], F16, tag="rcp")
            nc.vector.tensor_sub(denall[:], denall[:], npadr[:])
            with nc.allow_low_precision(
                    reason="fp16 softmax rcp; error budget validated"):
                nc.vector.reciprocal(rcp[:], denall[:])
            ps_rb = s3.enter_context(
                tc.tile_pool(name="ps_rb", bufs=1, space="PSUM"))
            rb = ps_rb.tile([128, 128], F32, tag="rb")
            nc.tensor.matmul(rb[:], onesrow[:], rcp[:], start=True, stop=True)
            nc.vector.tensor_mul(pvTn[:], pvraw[:], rb[:])

        # ---- phase 3: output projection ----------------------------------
        with ExitStack() as s5:
            outp = s5.enter_context(tc.tile_pool(name="outp", bufs=1))
            ps_o = s5.enter_context(
                tc.tile_pool(name="ps_o", bufs=8, space="PSUM"))
            out_sb = outp.tile([B, D], F32, tag="out")
            o_ps = [ps_o.tile([B, 512], F32, tag="ps_o", name=f"ops{n}")
                    for n in range(8)]
            pvr = pvTn[:].rearrange("p (b g) -> p b g", g=G)
            for g in range(G):
                lt = pvr[:, :, g]
                wo_t = wo_tiles[g // 2]
                for n in range(8):
                    nc.tensor.matmul(
                        o_ps[n][:], lt,
                        wo_t[:, (g % 2) * D + n * 512:(g % 2) * D
                             + (n + 1) * 512],
                        start=(g == 0), stop=(g == G - 1))
            for n in range(8):
                nc.vector.tensor_copy(out_sb[:, n * 512:(n + 1) * 512],
                                      o_ps[n][:])
            nc.sync.dma_start(out_d[:], out_sb[:])

    nc.compile()
    return nc


def _pack_kv(key_cache, value_cache, bt, Ls, h, pack_plan, totc):
    """Pack this head's valid context rows into the chunked KV stream."""
    kv = np.zeros((128, totc), dtype=np.float16)
    for b in range(B):
        _, base, nch = pack_plan[b]
        lg = Ls[b] - 1
        t = np.arange(lg, dtype=np.int64)
        slots = bt[b, t >> 4] * 16 + (t & 15)
        K = key_cache[slots, h, :]      # [lg, 128]
        V = value_cache[slots, h, :]    # [lg, 128]
        npad = nch * 128
        KT = np.zeros((128, npad), dtype=np.float32)
        KT[:, 0:lg] = K.T
        Vp = np.zeros((npad, 128), dtype=np.float32)
        Vp[0:lg, :] = V
        buf = np.empty((128, nch, CHW), dtype=np.float16)
        buf[:, :, 0:128] = KT.reshape(128, nch, 128)
        buf[:, :, 128:256] = Vp.reshape(nch, 128, 128).transpose(1, 0, 2)
        kv[:, base:base + nch * CHW] = buf.reshape(128, nch * CHW)
    return kv


def kernel(x, Wq, Wk, Wv, Wo, key_cache, value_cache, block_tables,
           context_lens):
    global LAST_RESULTS
    x = np.asarray(x, dtype=np.float32).reshape(B, D)
    xT = np.ascontiguousarray(
        x.reshape(B, 32, 128).transpose(2, 1, 0).reshape(128, 32 * B)
    ).astype(np.float16)
    Wq = np.asarray(Wq, dtype=np.float32)
    Wk = np.asarray(Wk, dtype=np.float32)
    Wv = np.asarray(Wv, dtype=np.float32)
    Wo = np.asarray(Wo, dtype=np.float32)
    key_cache = np.asarray(key_cache, dtype=np.float32)
    value_cache = np.asarray(value_cache, dtype=np.float32)
    bt = np.asarray(block_tables, dtype=np.int64)
    cl = np.asarray(context_lens, dtype=np.int64)

    Ls = [int(v) for v in cl]
    pos = np.array([v - 1 for v in Ls], dtype=np.int64)

    req_plan, tiles_plan = _plan(Ls)
    totc = tiles_plan[-1][0] + tiles_plan[-1][1]
    pack_plan = [(req_plan[b][0], tiles_plan[req_plan[b][0]][0]
                  + req_plan[b][1], req_plan[b][2]) for b in range(B)]

    # rope tables at the new token's position
    half = HEAD_DIM // 2
    inv_freq = (1.0 / (ROPE_BASE ** (np.arange(half, dtype=np.float32) / half))
                ).astype(np.float32)
    ang = pos.astype(np.float32)[:, None] * inv_freq[None, :]
    cqB = np.ascontiguousarray(np.cos(ang).astype(np.float32))  # [B, 64]
    sqB = np.ascontiguousarray(np.sin(ang).astype(np.float32))
    identH = np.eye(B, dtype=np.float16)
    npad = np.zeros((1, 128), dtype=np.float32)
    for b in range(B):
        npad[0, G * b:G * (b + 1)] = req_plan[b][2] * 128 - Ls[b]

    nc = _build_nc(Ls, req_plan, tiles_plan, totc)

    in_maps = []
    for h in range(N_CORES):
        wq_h = np.ascontiguousarray(
            Wq[:, h * GD:(h + 1) * GD].reshape(32, 128, GD)
            .transpose(1, 0, 2).reshape(128, 32 * GD)).astype(np.float16)
        wk_s = Wk[:, h * 128:(h + 1) * 128].reshape(32, 128, 128)
        wv_s = Wv[:, h * 128:(h + 1) * 128].reshape(32, 128, 128)
        wkv_h = np.ascontiguousarray(
            np.concatenate([wk_s, wv_s], axis=2)
            .transpose(1, 0, 2).reshape(128, 32 * 256)).astype(np.float16)
        wo_h = np.ascontiguousarray(
            Wo[h * GD:(h + 1) * GD, :].reshape(G, 128, D)
            .transpose(1, 0, 2).reshape(128, G * D)).astype(np.float16)
        kv_h = _pack_kv(key_cache, value_cache, bt, Ls, h, pack_plan, totc)
        in_maps.append({
            "xT": xT, "wq": wq_h, "wkv": wkv_h, "wo": wo_h, "kv": kv_h,
            "cq": cqB, "sq": sqB, "ident": identH, "npad": npad,
        })

    res = run_bass_kernel_spmd(nc, in_maps, list(range(N_CORES)))
    LAST_RESULTS = res

    out = np.zeros((B, D), dtype=np.float64)
    for h in range(N_CORES):
        out += res.results[h]["out"]
    return np.ascontiguousarray(out.reshape(B, 1, D).astype(np.float32))


# revision 28
# speedup vs baseline: 1.0584x; 1.0584x over previous
"""Paged-attention decode kernel for Trainium2 (Bass/Tile), 8 NeuronCores.

Sharding: one KV head per core (N_KV=8). Each core gets x^T plus its head's
slices of Wq/Wk/Wv/Wo (pre-transposed to DMA-friendly layouts, fp16) and a
host-packed KV stream, computes its 4 query heads' attention and a partial
output projection [B, D]; the host sums the partials.

The KV stream holds only the valid context rows, padded to 128-row chunks.
Chunk layout (256 cols): [K^T (128 cols, partition=d) | V (128 cols,
partition=t%128)].  The whole stream is fetched with a handful of multi-MB
contiguous DMAs into a rotating pool of SBUF tiles.

Per request b with nch chunks:
  QK:  per chunk, K^T chunk is the stationary operand (full 128 cols ->
       fast weight load), q [128,4] moving -> scores [t, g] in PSUM.
  exp: one activation over [128, 4*nch]; garbage rows of the partial last
       chunk are zeroed with a tiny memset so later sums are exact.
  PV:  per chunk, V chunk is stationary (fast weight load), exp-scores
       [128,4] moving -> accumulates att^T [d, g] directly in PSUM (no
       transpose needed later).
  den: one matmul (ones column stationary, scores moving) -> [1, 4*nch],
       then a strided DVE reduce over chunks -> denominators [1,4] written
       into a per-request slice of a shared row.
Normalization is batched at the end: one reciprocal [1,128], one
broadcast matmul (ones-row x rcp -> [128,128]), one elementwise multiply.

The new token's k/v never touch DRAM: its slot inside the last chunk is
patched on device (DVE column copy for K^T, one tiny DMA for the V row).

Everything on the wire is fp16 (measured end-to-end error vs the fp32
reference: ~6e-4); accumulation stays fp32 in PSUM.
"""
import os
import sys
from contextlib import ExitStack

import numpy as np

for _p in ("/opt/trn_rl_repo", "/opt/pypackages"):
    if os.path.isdir(_p) and _p not in sys.path:
        sys.path.append(_p)

import concourse.bass as bass  # noqa: E402,F401
import concourse.tile as tile  # noqa: E402
from concourse import bacc, mybir  # noqa: E402
from concourse.bass_utils import run_bass_kernel_spmd  # noqa: E402

N_HEADS = 32
N_KV = 8
HEAD_DIM = 128
BLOCK_SIZE = 16
MAX_SEQ = 2048
ROPE_BASE = 10000.0
SCALE = HEAD_DIM ** -0.5
B = 32
D = 4096
G = N_HEADS // N_KV   # 4 query heads per kv head
GD = G * HEAD_DIM     # 512
N_CORES = 8
CHW = 2 * HEAD_DIM              # chunk width in the packed KV stream (256)
TILE_CHUNKS = 48                # chunks per SBUF tile
TILE_COLS = TILE_CHUNKS * CHW   # 12288 cols (24 KiB/partition fp16)

F32 = mybir.dt.float32
F16 = mybir.dt.float16

LAST_RESULTS = None  # test harness reads exec_time_ns from here


def _plan(Ls):
    """Greedy-pack requests (in order) into KV tiles of <= TILE_CHUNKS
    chunks. Returns per-request (tile, base_col, nch) and per-tile
    (src_col, cols)."""
    req = []      # b -> (tile, base, nch)
    tiles = []    # tile -> (src_col, cols)
    cur_cols = 0
    src = 0
    for b in range(B):
        nch = (Ls[b] + 127) // 128  # chunks incl. the new-token slot
        w = nch * CHW
        if cur_cols + w > TILE_COLS and cur_cols > 0:
            tiles.append((src, cur_cols))
            src += cur_cols
            cur_cols = 0
        req.append((len(tiles), cur_cols, nch))
        cur_cols += w
    tiles.append((src, cur_cols))
    return req, tiles


def _build_nc(Ls, req_plan, tiles_plan, totc):
    nc = bacc.Bacc("TRN2", target_bir_lowering=False, debug=False,
                   num_devices=N_CORES)

    xt_d = nc.declare_dram_parameter("xT", [128, 32 * B], F16, isOutput=False)
    wq_d = nc.declare_dram_parameter("wq", [128, 32 * GD], F16, isOutput=False)
    wkv_d = nc.declare_dram_parameter("wkv", [128, 32 * 256], F16,
                                      isOutput=False)
    wo_d = nc.declare_dram_parameter("wo", [128, G * D], F16, isOutput=False)
    kv_d = nc.declare_dram_parameter("kv", [128, totc], F16, isOutput=False)
    cq_d = nc.declare_dram_parameter("cq", [B, 64], F32, isOutput=False)
    sq_d = nc.declare_dram_parameter("sq", [B, 64], F32, isOutput=False)
    npad_d = nc.declare_dram_parameter("npad", [1, 128], F32, isOutput=False)
    id_d = nc.declare_dram_parameter("ident", [B, B], F16, isOutput=False)
    out_d = nc.declare_dram_parameter("out", [B, D], F32, isOutput=True)

    with tile.TileContext(nc) as tc, ExitStack() as top:
        cpool = top.enter_context(tc.tile_pool(name="const", bufs=1))
        qT = cpool.tile([128, G * B], F16, tag="qT")     # [d, g*32+b] roped
        knT = cpool.tile([128, B], F16, tag="knT")       # [d, b] roped new k
        vn = cpool.tile([B, 128], F16, tag="vn")         # [b, d] new v
        onescol = cpool.tile([128, 1], F16, tag="ocol")
        onesrow = cpool.tile([1, 128], F16, tag="orow")
        denall = cpool.tile([1, 128], F32, tag="denall")  # [1, b*4+g]
        npadr = cpool.tile([1, 128], F32, tag="npad")
        nc.gpsimd.dma_start(npadr[:], npad_d[:])
        pvraw = cpool.tile([128, 128], F16, tag="pvraw")  # [d, b*4+g] unnorm
        pvTn = cpool.tile([128, 128], F16, tag="pvTn")    # [d, b*4+g] normed
        identH = cpool.tile([B, B], F16, tag="ident")
        nc.vector.memset(onescol[:], 1.0)
        nc.vector.memset(onesrow[:], 1.0)
        nc.gpsimd.dma_start(identH[:], id_d[:])

        kvpool = top.enter_context(tc.tile_pool(name="KV", bufs=4))
        scpool = top.enter_context(tc.tile_pool(name="SC", bufs=3))
        wop = top.enter_context(tc.tile_pool(name="wo", bufs=2))
        kv_tiles = {}
        wo_tiles = []

        def emit_kv(t):
            # split each tile across both HWDGE queues so two DMAs are in
            # flight and their completion latencies overlap
            src, cols = tiles_plan[t]
            kvt = kvpool.tile([128, TILE_COLS], F16, tag="kv", name=f"kv{t}")
            h = (cols // 2) // CHW * CHW
            nc.sync.dma_start(kvt[:, 0:h], kv_d[:, src:src + h])
            nc.scalar.dma_start(kvt[:, h:cols], kv_d[:, src + h:src + cols])
            kv_tiles[t] = kvt

        def emit_wo(i):
            wo_t = wop.tile([128, 2 * D], F16, tag="wo", name=f"wo{i}")
            nc.gpsimd.dma_start(wo_t[:], wo_d[:, i * 2 * D:(i + 1) * 2 * D])
            wo_tiles.append(wo_t)

        # ---- phase 1: q/k/v projections + rope (row layout [b, d]) -------
        # DMA queues: sync carries xT + the first half of Wq + the KV
        # stream; scalar carries cq/sq/Wkv + the rest of Wq (+ Wo later);
        # gpsimd carries tiny constants and, later, the patches.  K/V
        # projections run first so knT/vn (which gate the patches) are
        # ready early.
        with ExitStack() as s1:
            p1 = s1.enter_context(tc.tile_pool(name="p1", bufs=1))
            wqp = s1.enter_context(tc.tile_pool(name="wqp", bufs=8))
            ps1 = s1.enter_context(
                tc.tile_pool(name="ps1", bufs=1, space="PSUM"))
            tmp = s1.enter_context(tc.tile_pool(name="rtmp", bufs=4))

            xT = p1.tile([128, 32 * B], F16, tag="xT")   # [d, kc*32+b]
            nc.sync.dma_start(xT[:], xt_d[:])
            wq_tiles = []
            for i in range(8):
                wq_t = wqp.tile([128, 4 * GD], F16, tag="wq", name=f"wq{i}")
                eng = nc.sync if i < 4 else nc.scalar
                if i == 4:
                    cq = p1.tile([B, 64], F32, tag="cq")
                    sq = p1.tile([B, 64], F32, tag="sq")
                    nc.scalar.dma_start(cq[:], cq_d[:])
                    nc.scalar.dma_start(sq[:], sq_d[:])
                    wkv_sb = p1.tile([128, 32 * 256], F16, tag="wkv")
                    nc.scalar.dma_start(wkv_sb[:], wkv_d[:])
                eng.dma_start(wq_t[:], wq_d[:, i * 4 * GD:(i + 1) * 4 * GD])
                wq_tiles.append(wq_t)
            emit_kv(0)
            if len(tiles_plan) > 1:
                emit_kv(1)

            q_ps = ps1.tile([B, GD], F32, tag="ps_q")     # [b, g*128+d]
            kv_ps = ps1.tile([B, 256], F32, tag="ps_kv")  # [b, k|v]

            # rope in row layout: cols [0:64] x1, [64:128] x2 per head
            def rope_row(src, o0, o1):
                t1 = tmp.tile([B, 64], F32, tag="rt1", name="t1")
                t2 = tmp.tile([B, 64], F32, tag="rt2", name="t2")
                nc.vector.tensor_mul(t1[:], src[:, 0:64], cq[:])
                nc.vector.tensor_mul(t2[:], src[:, 64:128], sq[:])
                nc.vector.tensor_sub(o0, t1[:], t2[:])
                t3 = tmp.tile([B, 64], F32, tag="rt1", name="t3")
                t4 = tmp.tile([B, 64], F32, tag="rt2", name="t4")
                nc.vector.tensor_mul(t3[:], src[:, 0:64], sq[:])
                nc.vector.tensor_mul(t4[:], src[:, 64:128], cq[:])
                nc.vector.tensor_add(o1, t3[:], t4[:])

            qr = p1.tile([B, GD], F16, tag="qr")
            knr = p1.tile([B, 128], F16, tag="knr")
            ps_t = s1.enter_context(
                tc.tile_pool(name="ps_t", bufs=1, space="PSUM"))

            # a few q matmuls first (their weights land earliest), then the
            # k/v projections (whose outputs gate the patches), then the
            # rest of q
            for kc in range(8):
                rx = xT[:, kc * B:(kc + 1) * B]
                nc.tensor.matmul(q_ps[:],
                                 rx, wq_tiles[kc // 4][:, (kc % 4) * GD:
                                                       (kc % 4 + 1) * GD],
                                 start=(kc == 0), stop=False)
            for kc in range(32):
                rx = xT[:, kc * B:(kc + 1) * B]
                nc.tensor.matmul(kv_ps[:],
                                 rx, wkv_sb[:, kc * 256:(kc + 1) * 256],
                                 start=(kc == 0), stop=(kc == 31))
            rope_row(kv_ps[:, 0:128], knr[:, 0:64], knr[:, 64:128])
            nc.vector.tensor_copy(vn[:], kv_ps[:, 128:256])
            knT_ps = ps_t.tile([128, B], F16, tag="ps_knT")
            nc.tensor.transpose(knT_ps[:], knr[:], identH[:])
            nc.vector.tensor_copy(knT[:], knT_ps[:])
            for kc in range(8, 32):
                rx = xT[:, kc * B:(kc + 1) * B]
                nc.tensor.matmul(q_ps[:],
                                 rx, wq_tiles[kc // 4][:, (kc % 4) * GD:
                                                       (kc % 4 + 1) * GD],
                                 start=False, stop=(kc == 31))
            for g in range(G):
                rope_row(q_ps[:, g * 128:(g + 1) * 128],
                         qr[:, g * 128:g * 128 + 64],
                         qr[:, g * 128 + 64:(g + 1) * 128])

            # transpose q to [d, b] layout for the attention matmuls
            qT_ps = ps_t.tile([128, 128], F16, tag="ps_qT")
            for g in range(G):
                nc.tensor.transpose(qT_ps[:, g * B:(g + 1) * B],
                                    qr[:, g * 128:(g + 1) * 128],
                                    identH[:])
            nc.vector.tensor_copy(qT[:], qT_ps[:])

        # ---- phase 2: per-request attention ------------------------------
        with ExitStack() as s3:
            ps_qk = s3.enter_context(
                tc.tile_pool(name="ps_qk", bufs=2, space="PSUM"))
            ps_pv = s3.enter_context(
                tc.tile_pool(name="ps_pv", bufs=3, space="PSUM"))
            ps_d = s3.enter_context(
                tc.tile_pool(name="ps_d", bufs=2, space="PSUM"))

            qks = {}
            rqv = qT[:].rearrange("p (g b) -> p g b", b=B)

            def emit_patch_k(b):
                t, base, nch = req_plan[b]
                kvt = kv_tiles[t]
                lg = Ls[b] - 1
                cb = base + (nch - 1) * CHW
                rnew = lg % 128
                nc.vector.tensor_copy(kvt[:, cb + rnew:cb + rnew + 1],
                                      knT[:, b:b + 1])

            def emit_patch_v(b):
                t, base, nch = req_plan[b]
                kvt = kv_tiles[t]
                lg = Ls[b] - 1
                cb = base + (nch - 1) * CHW
                rnew = lg % 128
                nc.gpsimd.dma_start(
                    kvt[rnew:rnew + 1, cb + 128:cb + 256],
                    vn[b:b + 1, :])

            def emit_qk(b):
                # Pad K columns are zero (plus the patched new-token col),
                # so pad rows score exp(0)=1 exactly; the denominator is
                # corrected once at the end by subtracting the pad counts.
                t, base, nch = req_plan[b]
                kvt = kv_tiles[t]
                rq = rqv[:, :, b]
                qk = ps_qk.tile([128, G * 16], F32, tag="ps_qk",
                                name=f"qk{b}")
                sc = scpool.tile([128, G * 16], F16, tag="SC", name=f"sc{b}")
                for c in range(nch):
                    nc.tensor.matmul(qk[0:128, c * G:(c + 1) * G],
                                     kvt[:, base + c * CHW:base + c * CHW
                                         + 128],
                                     rq, start=True, stop=True)
                nc.scalar.activation(sc[:, 0:G * nch], qk[:, 0:G * nch],
                                     mybir.ActivationFunctionType.Exp,
                                     scale=SCALE)
                qks[b] = sc

            def emit_pv(b):
                t, base, nch = req_plan[b]
                kvt = kv_tiles[t]
                sc = qks.pop(b)
                pv = ps_pv.tile([128, G], F32, tag="ps_pv", name=f"pv{b}")
                for c in range(nch):
                    nc.tensor.matmul(pv[:],
                                     kvt[:, base + c * CHW + 128:
                                         base + c * CHW + 256],
                                     sc[:, c * G:(c + 1) * G],
                                     start=(c == 0), stop=(c == nch - 1))
                d1 = ps_d.tile([1, G * 16], F32, tag="ps_d", name=f"d1{b}")
                nc.tensor.matmul(d1[:, 0:G * nch], onescol[:],
                                 sc[:, 0:G * nch], start=True, stop=True)
                nc.vector.tensor_reduce(
                    denall[:, G * b:G * (b + 1)],
                    d1[:, 0:G * nch].rearrange("p (c g) -> p g c", g=G),
                    mybir.AxisListType.X, mybir.AluOpType.add)
                nc.vector.tensor_copy(pvraw[:, G * b:G * (b + 1)], pv[:])

            # Tile t is processed while t+1..t+2 stream in (3-buf pool).
            # Patches for tile t+1 are emitted at the middle of tile t —
            # by then t+1's DMA has landed, so the in-order DVE/Q7 queues
            # don't stall on it, and tile-granular RAW deps are long
            # resolved when t+1's QKs begin.
            tile_reqs = {}
            for bb in range(B):
                tile_reqs.setdefault(req_plan[bb][0], []).append(bb)
            patched_tiles = set()

            def emit_patches(t):
                if t in patched_tiles or t not in tile_reqs:
                    return
                patched_tiles.add(t)
                for bp in tile_reqs[t]:
                    emit_patch_k(bp)
                for bp in tile_reqs[t]:
                    emit_patch_v(bp)

            cur_t = -1
            pending = []
            for b in range(B):
                t = req_plan[b][0]
                if t > cur_t:
                    if t + 2 < len(tiles_plan) and (t + 2) not in kv_tiles:
                        emit_kv(t + 2)
                    emit_patches(t)  # no-op except for tile 0
                    cur_t = t
                if b in (10, 20):
                    emit_wo((b - 10) // 10)
                late = tile_reqs[t][max(0, len(tile_reqs[t]) - 2)]
                if b == late:
                    emit_patches(t + 1)
                emit_qk(b)
                pending.append(b)
                if len(pending) > 2:
                    emit_pv(pending.pop(0))
            while pending:
                emit_pv(pending.pop(0))

            # batched softmax normalization: pvTn = pvraw * (1/den) per col
            rcp = cpool.tile([1, 128], F16, tag="rcp")
            nc.vector.tensor_sub(denall[:], denall[:], npadr[:])
            with nc.allow_low_precision(
                    reason="fp16 softmax rcp; error budget validated"):
                nc.vector.reciprocal(rcp[:], denall[:])
            ps_rb = s3.enter_context(
                tc.tile_pool(name="ps_rb", bufs=1, space="PSUM"))
            rb = ps_rb.tile([128, 128], F32, tag="rb")
            nc.tensor.matmul(rb[:], onesrow[:], rcp[:], start=True, stop=True)
            nc.vector.tensor_mul(pvTn[:], pvraw[:], rb[:])

        # ---- phase 3: output projection ----------------------------------
        with ExitStack() as s5:
            outp = s5.enter_context(tc.tile_pool(name="outp", bufs=1))
            ps_o = s5.enter_context(
                tc.tile_pool(name="ps_o", bufs=8, space="PSUM"))
            out_sb = outp.tile([B, D], F32, tag="out")
            o_ps = [ps_o.tile([B, 512], F32, tag="ps_o", name=f"ops{n}")
                    for n in range(8)]
            pvr = pvTn[:].rearrange("p (b g) -> p b g", g=G)
            for g in range(G):
                lt = pvr[:, :, g]
                wo_t = wo_tiles[g // 2]
                for n in range(8):
                    nc.tensor.matmul(
                        o_ps[n][:], lt,
                        wo_t[:, (g % 2) * D + n * 512:(g % 2) * D
                             + (n + 1) * 512],
                        start=(g == 0), stop=(g == G - 1))
            for n in range(8):
                nc.vector.tensor_copy(out_sb[:, n * 512:(n + 1) * 512],
                                      o_ps[n][:])
            nc.sync.dma_start(out_d[:], out_sb[:])

    nc.compile()
    return nc


def _pack_kv(key_cache, value_cache, bt, Ls, h, pack_plan, totc):
    """Pack this head's valid context rows into the chunked KV stream."""
    kv = np.zeros((128, totc), dtype=np.float16)
    for b in range(B):
        _, base, nch = pack_plan[b]
        lg = Ls[b] - 1
        t = np.arange(lg, dtype=np.int64)
        slots = bt[b, t >> 4] * 16 + (t & 15)
        K = key_cache[slots, h, :]      # [lg, 128]
        V = value_cache[slots, h, :]    # [lg, 128]
        npad = nch * 128
        KT = np.zeros((128, npad), dtype=np.float32)
        KT[:, 0:lg] = K.T
        Vp = np.zeros((npad, 128), dtype=np.float32)
        Vp[0:lg, :] = V
        buf = np.empty((128, nch, CHW), dtype=np.float16)
        buf[:, :, 0:128] = KT.reshape(128, nch, 128)
        buf[:, :, 128:256] = Vp.reshape(nch, 128, 128).transpose(1, 0, 2)
        kv[:, base:base + nch * CHW] = buf.reshape(128, nch * CHW)
    return kv


def kernel(x, Wq, Wk, Wv, Wo, key_cache, value_cache, block_tables,
           context_lens):
    global LAST_RESULTS
    x = np.asarray(x, dtype=np.float32).reshape(B, D)
    xT = np.ascontiguousarray(
        x.reshape(B, 32, 128).transpose(2, 1, 0).reshape(128, 32 * B)
    ).astype(np.float16)
    Wq = np.asarray(Wq, dtype=np.float32)
    Wk = np.asarray(Wk, dtype=np.float32)
    Wv = np.asarray(Wv, dtype=np.float32)
    Wo = np.asarray(Wo, dtype=np.float32)
    key_cache = np.asarray(key_cache, dtype=np.float32)
    value_cache = np.asarray(value_cache, dtype=np.float32)
    bt = np.asarray(block_tables, dtype=np.int64)
    cl = np.asarray(context_lens, dtype=np.int64)

    Ls = [int(v) for v in cl]
    pos = np.array([v - 1 for v in Ls], dtype=np.int64)

    req_plan, tiles_plan = _plan(Ls)
    totc = tiles_plan[-1][0] + tiles_plan[-1][1]
    pack_plan = [(req_plan[b][0], tiles_plan[req_plan[b][0]][0]
                  + req_plan[b][1], req_plan[b][2]) for b in range(B)]

    # rope tables at the new token's position
    half = HEAD_DIM // 2
    inv_freq = (1.0 / (ROPE_BASE ** (np.arange(half, dtype=np.float32) / half))
                ).astype(np.float32)
    ang = pos.astype(np.float32)[:, None] * inv_freq[None, :]
    cqB = np.ascontiguousarray(np.cos(ang).astype(np.float32))  # [B, 64]
    sqB = np.ascontiguousarray(np.sin(ang).astype(np.float32))
    identH = np.eye(B, dtype=np.float16)
    npad = np.zeros((1, 128), dtype=np.float32)
    for b in range(B):
        npad[0, G * b:G * (b + 1)] = req_plan[b][2] * 128 - Ls[b]

    nc = _build_nc(Ls, req_plan, tiles_plan, totc)

    in_maps = []
    for h in range(N_CORES):
        wq_h = np.ascontiguousarray(
            Wq[:, h * GD:(h + 1) * GD].reshape(32, 128, GD)
            .transpose(1, 0, 2).reshape(128, 32 * GD)).astype(np.float16)
        wk_s = Wk[:, h * 128:(h + 1) * 128].reshape(32, 128, 128)
        wv_s = Wv[:, h * 128:(h + 1) * 128].reshape(32, 128, 128)
        wkv_h = np.ascontiguousarray(
            np.concatenate([wk_s, wv_s], axis=2)
            .transpose(1, 0, 2).reshape(128, 32 * 256)).astype(np.float16)
        wo_h = np.ascontiguousarray(
            Wo[h * GD:(h + 1) * GD, :].reshape(G, 128, D)
            .transpose(1, 0, 2).reshape(128, G * D)).astype(np.float16)
        kv_h = _pack_kv(key_cache, value_cache, bt, Ls, h, pack_plan, totc)
        in_maps.append({
            "xT": xT, "wq": wq_h, "wkv": wkv_h, "wo": wo_h, "kv": kv_h,
            "cq": cqB, "sq": sqB, "ident": identH, "npad": npad,
        })

    res = run_bass_kernel_spmd(nc, in_maps, list(range(N_CORES)))
    LAST_RESULTS = res

    out = np.zeros((B, D), dtype=np.float64)
    for h in range(N_CORES):
        out += res.results[h]["out"]
    return np.ascontiguousarray(out.reshape(B, 1, D).astype(np.float32))


# revision 36
# speedup vs baseline: 1.1039x; 1.0430x over previous
"""Paged-attention decode kernel for Trainium2 (Bass/Tile), 8 NeuronCores.

Sharding: one KV head per core (N_KV=8). Each core gets x^T plus its head's
slices of Wq/Wk/Wv/Wo (pre-transposed to DMA-friendly layouts, fp16) and a
host-packed KV stream, computes its 4 query heads' attention and a partial
output projection [B, D]; the host sums the partials.

The KV stream holds only the valid context rows, padded to 128-row chunks.
Chunk layout (256 cols): [K^T (128 cols, partition=d) | V (128 cols,
partition=t%128)].  The whole stream is fetched with a handful of multi-MB
contiguous DMAs into a rotating pool of SBUF tiles.

Per request b with nch chunks:
  QK:  per chunk, K^T chunk is the stationary operand (full 128 cols ->
       fast weight load), q [128,4] moving -> scores [t, g] in PSUM.
  exp: one activation over [128, 4*nch]; garbage rows of the partial last
       chunk are zeroed with a tiny memset so later sums are exact.
  PV:  per chunk, V chunk is stationary (fast weight load), exp-scores
       [128,4] moving -> accumulates att^T [d, g] directly in PSUM (no
       transpose needed later).
  den: one matmul (ones column stationary, scores moving) -> [1, 4*nch],
       then a strided DVE reduce over chunks -> denominators [1,4] written
       into a per-request slice of a shared row.
Normalization is batched at the end: one reciprocal [1,128], one
broadcast matmul (ones-row x rcp -> [128,128]), one elementwise multiply.

The new token's k/v never touch DRAM: its slot inside the last chunk is
patched on device (DVE column copy for K^T, one tiny DMA for the V row).

Everything on the wire is fp16 (measured end-to-end error vs the fp32
reference: ~6e-4); accumulation stays fp32 in PSUM.
"""
import os
import sys
from contextlib import ExitStack

import numpy as np

for _p in ("/opt/trn_rl_repo", "/opt/pypackages"):
    if os.path.isdir(_p) and _p not in sys.path:
        sys.path.append(_p)

import concourse.bass as bass  # noqa: E402,F401
import concourse.tile as tile  # noqa: E402
from concourse import bacc, mybir  # noqa: E402
from concourse.bass_utils import run_bass_kernel_spmd  # noqa: E402

N_HEADS = 32
N_KV = 8
HEAD_DIM = 128
BLOCK_SIZE = 16
MAX_SEQ = 2048
ROPE_BASE = 10000.0
SCALE = HEAD_DIM ** -0.5
B = 32
D = 4096
G = N_HEADS // N_KV   # 4 query heads per kv head
GD = G * HEAD_DIM     # 512
N_CORES = 8
CHW = 2 * HEAD_DIM              # chunk width in the packed KV stream (256)
TILE_CHUNKS = 48                # chunks per SBUF tile
TILE_COLS = TILE_CHUNKS * CHW   # 12288 cols (24 KiB/partition fp16)

F32 = mybir.dt.float32
F16 = mybir.dt.float16
F8E3 = mybir.dt.float8e3

LAST_RESULTS = None  # test harness reads exec_time_ns from here


def _plan(Ls):
    """Greedy-pack requests (in order) into KV tiles of <= TILE_CHUNKS
    chunks. Returns per-request (tile, base_col, nch) and per-tile
    (src_col, cols)."""
    req = []      # b -> (tile, base, nch)
    tiles = []    # tile -> (src_col, cols)
    cur_cols = 0
    src = 0
    for b in range(B):
        nch = (Ls[b] + 127) // 128  # chunks incl. the new-token slot
        w = nch * CHW
        if cur_cols + w > TILE_COLS and cur_cols > 0:
            tiles.append((src, cur_cols))
            src += cur_cols
            cur_cols = 0
        req.append((len(tiles), cur_cols, nch))
        cur_cols += w
    tiles.append((src, cur_cols))
    return req, tiles


def _build_nc(Ls, req_plan, tiles_plan, totc):
    nc = bacc.Bacc("TRN2", target_bir_lowering=False, debug=False,
                   num_devices=N_CORES)

    xt_d = nc.declare_dram_parameter("xT", [128, 32 * B], F16, isOutput=False)
    wq_d = nc.declare_dram_parameter("wq", [128, 32 * GD], F16, isOutput=False)
    wkv_d = nc.declare_dram_parameter("wkv", [128, 32 * 256], F16,
                                      isOutput=False)
    wo_d = nc.declare_dram_parameter("wo", [128, G * D], F16, isOutput=False)
    k_d = nc.declare_dram_parameter("k8", [128, totc // 2], F8E3,
                                    isOutput=False)
    v_d = nc.declare_dram_parameter("v16", [128, totc // 2], F16,
                                    isOutput=False)
    cq_d = nc.declare_dram_parameter("cq", [B, 64], F32, isOutput=False)
    sq_d = nc.declare_dram_parameter("sq", [B, 64], F32, isOutput=False)
    npad_d = nc.declare_dram_parameter("npad", [1, 128], F32, isOutput=False)
    id_d = nc.declare_dram_parameter("ident", [B, B], F16, isOutput=False)
    out_d = nc.declare_dram_parameter("out", [B, D], F32, isOutput=True)

    with tile.TileContext(nc) as tc, ExitStack() as top:
        cpool = top.enter_context(tc.tile_pool(name="const", bufs=1))
        qT = cpool.tile([128, G * B], F16, tag="qT")     # [d, g*32+b] roped
        knT = cpool.tile([128, B], F16, tag="knT")       # [d, b] roped new k
        vn = cpool.tile([B, 128], F16, tag="vn")         # [b, d] new v
        onescol = cpool.tile([128, 1], F16, tag="ocol")
        onesrow = cpool.tile([1, 128], F16, tag="orow")
        denall = cpool.tile([1, 128], F32, tag="denall")  # [1, b*4+g]
        npadr = cpool.tile([1, 128], F32, tag="npad")
        nc.gpsimd.dma_start(npadr[:], npad_d[:])
        pvraw = cpool.tile([128, 128], F16, tag="pvraw")  # [d, b*4+g] unnorm
        pvTn = cpool.tile([128, 128], F16, tag="pvTn")    # [d, b*4+g] normed
        identH = cpool.tile([B, B], F16, tag="ident")
        nc.vector.memset(onescol[:], 1.0)
        nc.vector.memset(onesrow[:], 1.0)
        nc.gpsimd.dma_start(identH[:], id_d[:])

        kvpool = top.enter_context(tc.tile_pool(name="KV", bufs=4))
        scpool = top.enter_context(tc.tile_pool(name="SC", bufs=3))
        wop = top.enter_context(tc.tile_pool(name="wo", bufs=2))
        kv_tiles = {}
        wo_tiles = []

        def emit_kv(t):
            # separate K (fp8) and V (fp16) streams, one per HWDGE queue,
            # so two DMAs are in flight and completion latencies overlap
            src, cols = tiles_plan[t]
            kt = kvpool.tile([128, TILE_COLS // 2], F8E3, tag="k",
                             name=f"k{t}")
            vt = kvpool.tile([128, TILE_COLS // 2], F16, tag="v",
                             name=f"v{t}")
            nc.sync.dma_start(kt[:, 0:cols // 2],
                              k_d[:, src // 2:(src + cols) // 2])
            nc.scalar.dma_start(vt[:, 0:cols // 2],
                                v_d[:, src // 2:(src + cols) // 2])
            kv_tiles[t] = (kt, vt)

        def emit_wo(i):
            wo_t = wop.tile([128, 2 * D], F16, tag="wo", name=f"wo{i}")
            nc.gpsimd.dma_start(wo_t[:], wo_d[:, i * 2 * D:(i + 1) * 2 * D])
            wo_tiles.append(wo_t)

        # ---- phase 1: q/k/v projections + rope (row layout [b, d]) -------
        # DMA queues: sync carries xT + the first half of Wq + the KV
        # stream; scalar carries cq/sq/Wkv + the rest of Wq (+ Wo later);
        # gpsimd carries tiny constants and, later, the patches.  K/V
        # projections run first so knT/vn (which gate the patches) are
        # ready early.
        with ExitStack() as s1:
            p1 = s1.enter_context(tc.tile_pool(name="p1", bufs=1))
            wqp = s1.enter_context(tc.tile_pool(name="wqp", bufs=8))
            ps1 = s1.enter_context(
                tc.tile_pool(name="ps1", bufs=1, space="PSUM"))
            tmp = s1.enter_context(tc.tile_pool(name="rtmp", bufs=4))

            xT = p1.tile([128, 32 * B], F16, tag="xT")   # [d, kc*32+b]
            nc.sync.dma_start(xT[:], xt_d[:])
            wq_tiles = []
            for i in range(8):
                wq_t = wqp.tile([128, 4 * GD], F16, tag="wq", name=f"wq{i}")
                eng = nc.sync if i < 4 else nc.scalar
                if i == 4:
                    cq = p1.tile([B, 64], F32, tag="cq")
                    sq = p1.tile([B, 64], F32, tag="sq")
                    nc.scalar.dma_start(cq[:], cq_d[:])
                    nc.scalar.dma_start(sq[:], sq_d[:])
                    wkv_sb = p1.tile([128, 32 * 256], F16, tag="wkv")
                    nc.scalar.dma_start(wkv_sb[:], wkv_d[:])
                eng.dma_start(wq_t[:], wq_d[:, i * 4 * GD:(i + 1) * 4 * GD])
                wq_tiles.append(wq_t)
            emit_kv(0)
            if len(tiles_plan) > 1:
                emit_kv(1)

            q_ps = ps1.tile([B, GD], F32, tag="ps_q")     # [b, g*128+d]
            kv_ps = ps1.tile([B, 256], F32, tag="ps_kv")  # [b, k|v]

            # rope in row layout: cols [0:64] x1, [64:128] x2 per head
            def rope_row(src, o0, o1):
                t1 = tmp.tile([B, 64], F32, tag="rt1", name="t1")
                t2 = tmp.tile([B, 64], F32, tag="rt2", name="t2")
                nc.vector.tensor_mul(t1[:], src[:, 0:64], cq[:])
                nc.vector.tensor_mul(t2[:], src[:, 64:128], sq[:])
                nc.vector.tensor_sub(o0, t1[:], t2[:])
                t3 = tmp.tile([B, 64], F32, tag="rt1", name="t3")
                t4 = tmp.tile([B, 64], F32, tag="rt2", name="t4")
                nc.vector.tensor_mul(t3[:], src[:, 0:64], sq[:])
                nc.vector.tensor_mul(t4[:], src[:, 64:128], cq[:])
                nc.vector.tensor_add(o1, t3[:], t4[:])

            qr = p1.tile([B, GD], F16, tag="qr")
            knr = p1.tile([B, 128], F16, tag="knr")
            ps_t = s1.enter_context(
                tc.tile_pool(name="ps_t", bufs=1, space="PSUM"))

            # a few q matmuls first (their weights land earliest), then the
            # k/v projections (whose outputs gate the patches), then the
            # rest of q
            for kc in range(8):
                rx = xT[:, kc * B:(kc + 1) * B]
                nc.tensor.matmul(q_ps[:],
                                 rx, wq_tiles[kc // 4][:, (kc % 4) * GD:
                                                       (kc % 4 + 1) * GD],
                                 start=(kc == 0), stop=False)
            for kc in range(32):
                rx = xT[:, kc * B:(kc + 1) * B]
                nc.tensor.matmul(kv_ps[:],
                                 rx, wkv_sb[:, kc * 256:(kc + 1) * 256],
                                 start=(kc == 0), stop=(kc == 31))
            rope_row(kv_ps[:, 0:128], knr[:, 0:64], knr[:, 64:128])
            nc.vector.tensor_copy(vn[:], kv_ps[:, 128:256])
            knT_ps = ps_t.tile([128, B], F16, tag="ps_knT")
            nc.tensor.transpose(knT_ps[:], knr[:], identH[:])
            nc.vector.tensor_copy(knT[:], knT_ps[:])
            for kc in range(8, 32):
                rx = xT[:, kc * B:(kc + 1) * B]
                nc.tensor.matmul(q_ps[:],
                                 rx, wq_tiles[kc // 4][:, (kc % 4) * GD:
                                                       (kc % 4 + 1) * GD],
                                 start=False, stop=(kc == 31))
            for g in range(G):
                rope_row(q_ps[:, g * 128:(g + 1) * 128],
                         qr[:, g * 128:g * 128 + 64],
                         qr[:, g * 128 + 64:(g + 1) * 128])

            # transpose q to [d, b] layout for the attention matmuls
            qT_ps = ps_t.tile([128, 128], F16, tag="ps_qT")
            for g in range(G):
                nc.tensor.transpose(qT_ps[:, g * B:(g + 1) * B],
                                    qr[:, g * 128:(g + 1) * 128],
                                    identH[:])
            nc.vector.tensor_copy(qT[:], qT_ps[:])

        # ---- phase 2: per-request attention ------------------------------
        with ExitStack() as s3:
            ps_qk = s3.enter_context(
                tc.tile_pool(name="ps_qk", bufs=2, space="PSUM"))
            ps_pv = s3.enter_context(
                tc.tile_pool(name="ps_pv", bufs=3, space="PSUM"))
            ps_d = s3.enter_context(
                tc.tile_pool(name="ps_d", bufs=2, space="PSUM"))

            qks = {}
            rqv = qT[:].rearrange("p (g b) -> p g b", b=B)

            def emit_patch_k(b):
                t, base, nch = req_plan[b]
                kt, _ = kv_tiles[t]
                lg = Ls[b] - 1
                cb = base // 2 + (nch - 1) * 128
                rnew = lg % 128
                nc.vector.tensor_copy(kt[:, cb + rnew:cb + rnew + 1],
                                      knT[:, b:b + 1])

            def emit_patch_v(b):
                t, base, nch = req_plan[b]
                _, vt = kv_tiles[t]
                lg = Ls[b] - 1
                cb = base // 2 + (nch - 1) * 128
                rnew = lg % 128
                nc.gpsimd.dma_start(
                    vt[rnew:rnew + 1, cb:cb + 128],
                    vn[b:b + 1, :])

            def emit_qk(b):
                # Pad K columns are zero (plus the patched new-token col),
                # so pad rows score exp(0)=1 exactly; the denominator is
                # corrected once at the end by subtracting the pad counts.
                t, base, nch = req_plan[b]
                kt, _ = kv_tiles[t]
                kb = base // 2
                rq = rqv[:, :, b]
                qk = ps_qk.tile([128, G * 16], F32, tag="ps_qk",
                                name=f"qk{b}")
                sc = scpool.tile([128, G * 16], F16, tag="SC", name=f"sc{b}")
                for c in range(nch):
                    nc.tensor.matmul(qk[0:128, c * G:(c + 1) * G],
                                     kt[:, kb + c * 128:kb + (c + 1) * 128],
                                     rq, start=True, stop=True)
                nc.scalar.activation(sc[:, 0:G * nch], qk[:, 0:G * nch],
                                     mybir.ActivationFunctionType.Exp,
                                     scale=SCALE)
                qks[b] = sc

            def emit_pv(b):
                t, base, nch = req_plan[b]
                _, vt = kv_tiles[t]
                kb = base // 2
                sc = qks.pop(b)
                pv = ps_pv.tile([128, G], F32, tag="ps_pv", name=f"pv{b}")
                for c in range(nch):
                    nc.tensor.matmul(pv[:],
                                     vt[:, kb + c * 128:kb + (c + 1) * 128],
                                     sc[:, c * G:(c + 1) * G],
                                     start=(c == 0), stop=(c == nch - 1))
                d1 = ps_d.tile([1, G * 16], F32, tag="ps_d", name=f"d1{b}")
                nc.tensor.matmul(d1[:, 0:G * nch], onescol[:],
                                 sc[:, 0:G * nch], start=True, stop=True)
                nc.vector.tensor_reduce(
                    denall[:, G * b:G * (b + 1)],
                    d1[:, 0:G * nch].rearrange("p (c g) -> p g c", g=G),
                    mybir.AxisListType.X, mybir.AluOpType.add)
                nc.vector.tensor_copy(pvraw[:, G * b:G * (b + 1)], pv[:])

            # Tile t is processed while t+1..t+2 stream in (3-buf pool).
            # Patches for tile t+1 are emitted at the middle of tile t —
            # by then t+1's DMA has landed, so the in-order DVE/Q7 queues
            # don't stall on it, and tile-granular RAW deps are long
            # resolved when t+1's QKs begin.
            tile_reqs = {}
            for bb in range(B):
                tile_reqs.setdefault(req_plan[bb][0], []).append(bb)
            patched_tiles = set()

            def emit_patches(t):
                if t in patched_tiles or t not in tile_reqs:
                    return
                patched_tiles.add(t)
                for bp in tile_reqs[t]:
                    emit_patch_k(bp)
                for bp in tile_reqs[t]:
                    emit_patch_v(bp)

            cur_t = -1
            pending = []
            for b in range(B):
                t = req_plan[b][0]
                if t > cur_t:
                    if t + 2 < len(tiles_plan) and (t + 2) not in kv_tiles:
                        emit_kv(t + 2)
                    emit_patches(t)  # no-op except for tile 0
                    cur_t = t
                if b in (10, 20):
                    emit_wo((b - 10) // 10)
                late = tile_reqs[t][max(0, len(tile_reqs[t]) - 2)]
                if b == late:
                    emit_patches(t + 1)
                emit_qk(b)
                pending.append(b)
                if len(pending) > 2:
                    emit_pv(pending.pop(0))
            while pending:
                emit_pv(pending.pop(0))

            # batched softmax normalization: pvTn = pvraw * (1/den) per col
            rcp = cpool.tile([1, 128], F16, tag="rcp")
            nc.vector.tensor_sub(denall[:], denall[:], npadr[:])
            with nc.allow_low_precision(
                    reason="fp16 softmax rcp; error budget validated"):
                nc.vector.reciprocal(rcp[:], denall[:])
            ps_rb = s3.enter_context(
                tc.tile_pool(name="ps_rb", bufs=1, space="PSUM"))
            rb = ps_rb.tile([128, 128], F32, tag="rb")
            nc.tensor.matmul(rb[:], onesrow[:], rcp[:], start=True, stop=True)
            nc.vector.tensor_mul(pvTn[:], pvraw[:], rb[:])

        # ---- phase 3: output projection ----------------------------------
        with ExitStack() as s5:
            outp = s5.enter_context(tc.tile_pool(name="outp", bufs=1))
            ps_o = s5.enter_context(
                tc.tile_pool(name="ps_o", bufs=8, space="PSUM"))
            out_sb = outp.tile([B, D], F32, tag="out")
            o_ps = [ps_o.tile([B, 512], F32, tag="ps_o", name=f"ops{n}")
                    for n in range(8)]
            pvr = pvTn[:].rearrange("p (b g) -> p b g", g=G)
            for g in range(G):
                lt = pvr[:, :, g]
                wo_t = wo_tiles[g // 2]
                for n in range(8):
                    nc.tensor.matmul(
                        o_ps[n][:], lt,
                        wo_t[:, (g % 2) * D + n * 512:(g % 2) * D
                             + (n + 1) * 512],
                        start=(g == 0), stop=(g == G - 1))
            for n in range(8):
                nc.vector.tensor_copy(out_sb[:, n * 512:(n + 1) * 512],
                                      o_ps[n][:])
            nc.sync.dma_start(out_d[:], out_sb[:])

    nc.compile()
    return nc


def _pack_kv(key_cache, value_cache, bt, Ls, h, pack_plan, totc):
    """Pack this head's valid context rows into the chunked K (fp8e3m4)
    and V (fp16) streams."""
    import ml_dtypes
    k8 = np.zeros((128, totc // 2), dtype=ml_dtypes.float8_e3m4)
    v16 = np.zeros((128, totc // 2), dtype=np.float16)
    for b in range(B):
        _, base, nch = pack_plan[b]
        kb = base // 2
        lg = Ls[b] - 1
        t = np.arange(lg, dtype=np.int64)
        slots = bt[b, t >> 4] * 16 + (t & 15)
        K = key_cache[slots, h, :]      # [lg, 128]
        V = value_cache[slots, h, :]    # [lg, 128]
        k8[:, kb:kb + lg] = K.T.astype(ml_dtypes.float8_e3m4)
        npad = nch * 128
        Vp = np.zeros((npad, 128), dtype=np.float16)
        Vp[0:lg, :] = V
        v16[:, kb:kb + npad] = (
            Vp.reshape(nch, 128, 128).transpose(1, 0, 2)
            .reshape(128, npad))
    return k8, v16


def kernel(x, Wq, Wk, Wv, Wo, key_cache, value_cache, block_tables,
           context_lens):
    global LAST_RESULTS
    x = np.asarray(x, dtype=np.float32).reshape(B, D)
    xT = np.ascontiguousarray(
        x.reshape(B, 32, 128).transpose(2, 1, 0).reshape(128, 32 * B)
    ).astype(np.float16)
    Wq = np.asarray(Wq, dtype=np.float32)
    Wk = np.asarray(Wk, dtype=np.float32)
    Wv = np.asarray(Wv, dtype=np.float32)
    Wo = np.asarray(Wo, dtype=np.float32)
    key_cache = np.asarray(key_cache, dtype=np.float32)
    value_cache = np.asarray(value_cache, dtype=np.float32)
    bt = np.asarray(block_tables, dtype=np.int64)
    cl = np.asarray(context_lens, dtype=np.int64)

    Ls = [int(v) for v in cl]
    pos = np.array([v - 1 for v in Ls], dtype=np.int64)

    req_plan, tiles_plan = _plan(Ls)
    totc = tiles_plan[-1][0] + tiles_plan[-1][1]
    pack_plan = [(req_plan[b][0], tiles_plan[req_plan[b][0]][0]
                  + req_plan[b][1], req_plan[b][2]) for b in range(B)]

    # rope tables at the new token's position
    half = HEAD_DIM // 2
    inv_freq = (1.0 / (ROPE_BASE ** (np.arange(half, dtype=np.float32) / half))
                ).astype(np.float32)
    ang = pos.astype(np.float32)[:, None] * inv_freq[None, :]
    cqB = np.ascontiguousarray(np.cos(ang).astype(np.float32))  # [B, 64]
    sqB = np.ascontiguousarray(np.sin(ang).astype(np.float32))
    identH = np.eye(B, dtype=np.float16)
    npad = np.zeros((1, 128), dtype=np.float32)
    for b in range(B):
        npad[0, G * b:G * (b + 1)] = req_plan[b][2] * 128 - Ls[b]

    nc = _build_nc(Ls, req_plan, tiles_plan, totc)

    in_maps = []
    for h in range(N_CORES):
        wq_h = np.ascontiguousarray(
            Wq[:, h * GD:(h + 1) * GD].reshape(32, 128, GD)
            .transpose(1, 0, 2).reshape(128, 32 * GD)).astype(np.float16)
        wk_s = Wk[:, h * 128:(h + 1) * 128].reshape(32, 128, 128)
        wv_s = Wv[:, h * 128:(h + 1) * 128].reshape(32, 128, 128)
        wkv_h = np.ascontiguousarray(
            np.concatenate([wk_s, wv_s], axis=2)
            .transpose(1, 0, 2).reshape(128, 32 * 256)).astype(np.float16)
        wo_h = np.ascontiguousarray(
            Wo[h * GD:(h + 1) * GD, :].reshape(G, 128, D)
            .transpose(1, 0, 2).reshape(128, G * D)).astype(np.float16)
        k8_h, v16_h = _pack_kv(key_cache, value_cache, bt, Ls, h, pack_plan,
                               totc)
        in_maps.append({
            "xT": xT, "wq": wq_h, "wkv": wkv_h, "wo": wo_h,
            "k8": k8_h, "v16": v16_h,
            "cq": cqB, "sq": sqB, "ident": identH, "npad": npad,
        })

    res = run_bass_kernel_spmd(nc, in_maps, list(range(N_CORES)))
    LAST_RESULTS = res

    out = np.zeros((B, D), dtype=np.float64)
    for h in range(N_CORES):
        out += res.results[h]["out"]
    return np.ascontiguousarray(out.reshape(B, 1, D).astype(np.float32))
